# revision 8
# baseline (speedup 1.0000x reference)
"""Trainium2 Bass kernel for BranchTeacherLayoutLoss (segment_reduce).

Strategy: shard by segment range (B=512 segments -> 64 per core, contiguous
member runs because segment_ids is sorted). Each core gathers its members'
embedding rows from the full table via SWDGE dma_gather (int16-indexed ->
<=32768-row chunks; <=1024 rows per call, rotating 4 SWDGE queues). Per
gathered 128-row group it computes inverse row norms (ACT/DVE split), builds
a scaled one-hot segment-selection matrix (batched broadcast tensor_tensor),
and accumulates per-segment direction sums with PE matmuls into PSUM.
Per-core [64,2] losses come back; the host sums them.

v2: warmup gather to absorb SWDGE first-call overhead, chunked idx loads so
gather 0 starts early, endgame metadata loaded late, bf16 compute, batched
one-hot build, Rsqrt fuse. The gather DMA span is descriptor-bound
(~20-28ns/desc/engine); compute fully hides under it.
"""
import sys
import types
import os as _os
import numpy as np
from contextlib import ExitStack

if '/opt/trn_rl_repo' not in sys.path:
    sys.path.insert(0, '/opt/trn_rl_repo')

import concourse.bass as bass
import concourse.tile as tile
from concourse import bacc, mybir
from concourse.bass_utils import run_bass_kernel_spmd

F32 = mybir.dt.float32
I16 = mybir.dt.int16
BF = mybir.dt.bfloat16
Alu = mybir.AluOpType
Act = mybir.ActivationFunctionType

N_CORES = 8
CHUNK = 32768          # int16 index reach per dma_gather call
CALL = 1024            # max indices per dma_gather (SWDGE ring capacity)
N_QUEUES = 4
ACT_PER_CALL = int(_os.environ.get('ACT_PER_CALL', '4'))
SKIP_COMPUTE = _os.environ.get('SKIP_COMPUTE', '0') == '1'
WARMUP = _os.environ.get('WARMUP', '1') == '1'


def _plan(member_indices, segment_ids, N, B):
    """Host-side index planning. Returns per-core index/segment layouts and
    the static call plan (shared across cores)."""
    spc = B // N_CORES
    nch = (N + CHUNK - 1) // CHUNK
    idx_all = np.asarray(member_indices).astype(np.int64)
    seg_all = np.asarray(segment_ids).astype(np.int64)

    cores = []
    counts_ck = np.zeros((N_CORES, nch), dtype=np.int64)
    for c in range(N_CORES):
        lo = np.searchsorted(seg_all, c * spc, side='left')
        hi = np.searchsorted(seg_all, (c + 1) * spc, side='left')
        idx = idx_all[lo:hi]
        seg = seg_all[lo:hi] - c * spc
        ck = idx // CHUNK
        order = np.argsort(ck, kind='stable')
        idx, seg, ck = idx[order], seg[order], ck[order]
        counts = np.bincount(seg, minlength=spc).astype(np.float32)
        cores.append({'idx': idx, 'seg': seg, 'ck': ck, 'counts': counts})
        counts_ck[c] = np.bincount(ck, minlength=nch)

    # static per-chunk padded sizes and call splits (identical across cores)
    g_k = []
    calls = []  # list of (chunk_idx, call_size)
    chunk_first_call = [0] * nch
    for k in range(nch):
        mx = int(counts_ck[:, k].max())
        gk = ((mx + 127) // 128) * 128 if mx > 0 else 0
        g_k.append(gk)
        chunk_first_call[k] = len(calls)
        rem = gk
        while rem > 0:
            g = min(CALL, rem)
            calls.append((k, g))
            rem -= g

    for c in range(N_CORES):
        d = cores[c]
        idx16_cols = []
        seg_cols = []
        for k in range(nch):
            gk = g_k[k]
            if gk == 0:
                continue
            sel = d['ck'] == k
            n = int(sel.sum())
            loc = (d['idx'][sel] - k * CHUNK).astype(np.int16)
            segk = d['seg'][sel].astype(np.float32)
            idx_pad = np.zeros(gk, dtype=np.int16)
            idx_pad[:n] = loc
            seg_pad = np.full(gk, float(spc), dtype=np.float32)
            seg_pad[:n] = segk
            # idx wrap is PER CALL: [i%16, call_off + i//16]
            pos = 0
            while pos < gk:
                g = min(CALL, gk - pos)
                idx16_cols.append(idx_pad[pos:pos + g].reshape(g // 16, 16).T)
                pos += g
            seg_cols.append(seg_pad.reshape(gk // 128, 128).T)
        d['idx16'] = np.tile(np.concatenate(idx16_cols, axis=1), (8, 1))
        d['segf'] = np.concatenate(seg_cols, axis=1)
    return cores, calls, spc, nch


def _build(N, D, B, calls, spc):
    """Build and compile the SPMD Bass program (identical across cores)."""
    n_groups = sum(g for _, g in calls) // 128
    t_idx = sum(g for _, g in calls) // 16

    nc = bacc.Bacc("TRN2", target_bir_lowering=False, debug=False,
                   num_devices=N_CORES, num_swdge_queues=N_QUEUES)
    emb = nc.dram_tensor("emb", [N, D], BF, kind="ExternalInput")
    idx_in = nc.dram_tensor("idx_in", [128, t_idx], I16, kind="ExternalInput")
    oh_in = nc.dram_tensor("oh_in", [128, n_groups * spc], BF, kind="ExternalInput")
    tc_in = nc.dram_tensor("tc_in", [spc, D], F32, kind="ExternalInput")
    tcoh_in = nc.dram_tensor("tcoh_in", [spc, 1], F32, kind="ExternalInput")
    rcnt_in = nc.dram_tensor("rcnt_in", [spc, 1], F32, kind="ExternalInput")
    loss_out = nc.dram_tensor("loss_out", [spc, 2], F32, kind="ExternalOutput")

    # idx column ranges per chunk (for split DMA loads)
    chunk_cols = {}
    coff = 0
    for k, gcall in calls:
        c0, c1 = chunk_cols.get(k, (coff, coff))
        chunk_cols[k] = (c0, coff + gcall // 16)
        coff += gcall // 16

    with tile.TileContext(nc) as tc_ctx, ExitStack() as ctx:
        meta = ctx.enter_context(tc_ctx.tile_pool(name="meta", bufs=1))
        gpool = ctx.enter_context(tc_ctx.tile_pool(name="gather", bufs=8))
        spool = ctx.enter_context(tc_ctx.tile_pool(name="small", bufs=3))
        qpool = ctx.enter_context(tc_ctx.tile_pool(name="scratch", bufs=3))
        ppool = ctx.enter_context(tc_ctx.tile_pool(name="psum", bufs=1, space="PSUM"))
        fpool = ctx.enter_context(tc_ctx.tile_pool(name="final", bufs=1))

        if WARMUP:
            # tiny gather ASAP: absorbs SWDGE cold-start before idx loads land
            widx = meta.tile([128, 8], I16)
            nc.gpsimd.memset(widx[:], 0)
            wgt = meta.tile([128, 1, D], BF)
            nc.gpsimd.dma_gather(wgt[:], emb.ap()[0:CHUNK, :],
                                 widx[:, 0:8], 128, 128, D, queue_num=0)

        idxt = meta.tile([128, t_idx], I16)
        for k in sorted(chunk_cols):
            c0, c1 = chunk_cols[k]
            nc.sync.dma_start(idxt[:, c0:c1], idx_in.ap()[:, c0:c1])
        # static 0/1 one-hot, host-precomputed; loaded per chunk's group range
        ohall = meta.tile([128, n_groups, spc], BF)
        for k in sorted(chunk_cols):
            c0, c1 = chunk_cols[k]
            g0, g1 = c0 * 16 // 128, c1 * 16 // 128
            nc.sync.dma_start(ohall[:, g0:g1, :],
                              oh_in.ap()[:, g0 * spc:g1 * spc])

        psumA = ppool.tile([spc, D], F32, space="PSUM")
        psumB = ppool.tile([spc, D], F32, space="PSUM")

        g_all = 0   # global group counter
        coff = 0    # idx tile column offset (int16 cols)
        n_calls = len(calls)
        last_even = n_calls - 1 - ((n_calls - 1) % 2 != 0)
        last_odd = n_calls - 1 - ((n_calls - 1) % 2 == 0)
        for ci, (k, gcall) in enumerate(calls):
            r0 = k * CHUNK
            rows = min(CHUNK, N - r0)
            w = gcall // 128  # groups in this call (<= 8)
            gt = gpool.tile([128, w, D], BF, tag="gt")
            nc.gpsimd.dma_gather(
                gt[:], emb.ap()[r0:r0 + rows, :],
                idxt[:, coff:coff + gcall // 16], gcall, gcall, D,
                queue_num=ci % N_QUEUES)
            coff += gcall // 16
            if SKIP_COMPUTE:
                g_all += w
                continue
            # row sum-of-squares: one batched ACT square + one DVE reduce
            sq3 = qpool.tile([128, 8, D], BF, tag="sq3")
            nc.scalar.activation(sq3[:, :w, :], gt[:, :, :], Act.Square)
            ss = spool.tile([128, 8], F32, tag="ss")
            nc.vector.tensor_reduce(ss[:, :w], sq3[:, :w, :],
                                    axis=mybir.AxisListType.X, op=Alu.add)
            nrm = spool.tile([128, 8], F32, tag="nrm")
            nc.scalar.sqrt(nrm[:, :w], ss[:, :w])
            inv = spool.tile([128, 8], BF, tag="inv")
            with nc.allow_low_precision(reason="bf16 inv-norm; error washes out in segment means"):
                nc.vector.reciprocal(inv[:, :w], nrm[:, :w])
            # scaled selection: sw = static_onehot * inv  (one DVE op)
            sw = spool.tile([128, 8, spc], BF, tag="sw")
            nc.vector.tensor_tensor(
                sw[:, :w, :], ohall[:, g_all:g_all + w, :],
                inv[:, :w, None].to_broadcast([128, w, spc]),
                op=Alu.mult)
            psum = psumA if (ci % 2 == 0) else psumB
            is_last_of_parity = ci == (last_even if ci % 2 == 0 else last_odd)
            for j in range(w):
                nc.tensor.matmul(psum[:], lhsT=sw[:, j, :], rhs=gt[:, j, :],
                                 start=(ci < 2 and j == 0),
                                 stop=(is_last_of_parity and j == w - 1))
            g_all += w

        # endgame metadata (loaded late; only needed now)
        tcv = meta.tile([spc, D], F32)
        nc.sync.dma_start(tcv[:], tc_in.ap()[:, :])
        tco = meta.tile([spc, 1], F32)
        nc.sync.dma_start(tco[:], tcoh_in.ap()[:, :])
        rcn = meta.tile([spc, 1], F32)
        nc.sync.dma_start(rcn[:], rcnt_in.ap()[:, :])

        # endgame: per-segment losses from psum sums
        sums = fpool.tile([spc, D], F32)
        if SKIP_COMPUTE:
            nc.vector.memset(sums[:], 1.0)
        elif len(calls) > 1:
            sumsB = fpool.tile([spc, D], F32)
            nc.vector.tensor_copy(sumsB[:], psumB[:])
            nc.vector.tensor_tensor(sums[:], psumA[:], sumsB[:], op=Alu.add)
        else:
            nc.vector.tensor_copy(sums[:], psumA[:])
        mean = fpool.tile([spc, D], F32)
        nc.vector.tensor_scalar(mean[:], sums[:], rcn[:], None, op0=Alu.mult)
        scr = fpool.tile([spc, D], F32)
        msq = fpool.tile([spc, 1], F32)
        nc.vector.scalar_tensor_tensor(out=scr[:], in0=mean[:], scalar=1.0,
                                       in1=mean[:], op0=Alu.mult,
                                       op1=Alu.mult, accum_out=msq[:])
        scr2 = fpool.tile([spc, D], F32)
        tcd = fpool.tile([spc, 1], F32)
        nc.vector.scalar_tensor_tensor(out=scr2[:], in0=mean[:], scalar=1.0,
                                       in1=tcv[:], op0=Alu.mult,
                                       op1=Alu.mult, accum_out=tcd[:])
        nrm2 = fpool.tile([spc, 1], F32)
        nc.scalar.sqrt(nrm2[:], msq[:])
        den = fpool.tile([spc, 1], F32)
        nc.vector.tensor_scalar(den[:], nrm2[:], 1e-12, None, op0=Alu.max)
        invd = fpool.tile([spc, 1], F32)
        nc.vector.reciprocal(invd[:], den[:])
        # closs = 1 - tcd*invd ; coh = 1 - msq*invd ; coloss = relu(coh - tcoh)
        t0 = fpool.tile([spc, 1], F32)
        nc.vector.tensor_tensor(t0[:], tcd[:], invd[:], op=Alu.mult)
        closs = fpool.tile([spc, 1], F32)
        nc.scalar.activation(closs[:], t0[:], Act.Copy, bias=1.0, scale=-1.0)
        t1 = fpool.tile([spc, 1], F32)
        nc.vector.tensor_tensor(t1[:], msq[:], invd[:], op=Alu.mult)
        coh = fpool.tile([spc, 1], F32)
        nc.scalar.activation(coh[:], t1[:], Act.Copy, bias=1.0, scale=-1.0)
        t2 = fpool.tile([spc, 1], F32)
        nc.vector.tensor_tensor(t2[:], coh[:], tco[:], op=Alu.subtract)
        coloss = fpool.tile([spc, 1], F32)
        nc.vector.tensor_scalar(coloss[:], t2[:], 0.0, None, op0=Alu.max)
        out2 = fpool.tile([spc, 2], F32)
        nc.vector.tensor_copy(out2[:, 0:1], closs[:])
        nc.vector.tensor_copy(out2[:, 1:2], coloss[:])
        nc.sync.dma_start(loss_out.ap()[:, :], out2[:])

    nc.compile()
    return nc


def _prepare(embeddings, teacher_centroids, teacher_cohesion,
             member_indices, segment_ids):
    import ml_dtypes
    emb = np.ascontiguousarray(
        np.asarray(embeddings, dtype=np.float32).astype(ml_dtypes.bfloat16))
    tcv = np.ascontiguousarray(np.asarray(teacher_centroids, dtype=np.float32))
    tcoh = np.asarray(teacher_cohesion, dtype=np.float32)
    N, D = emb.shape
    B = tcv.shape[0]
    cores, calls, spc, nch = _plan(member_indices, segment_ids, N, B)
    nc = _build(N, D, B, calls, spc)
    in_maps = []
    for c in range(N_CORES):
        d = cores[c]
        # static one-hot [128, n_groups, spc] from per-group segment ids
        oh = (d['segf'][:, :, None] ==
              np.arange(spc, dtype=np.float32)[None, None, :])
        oh = np.ascontiguousarray(
            oh.astype(ml_dtypes.bfloat16).reshape(128, -1))
        in_maps.append({
            "emb": emb,
            "idx_in": np.ascontiguousarray(d['idx16']),
            "oh_in": oh,
            "tc_in": np.ascontiguousarray(tcv[c * spc:(c + 1) * spc]),
            "tcoh_in": np.ascontiguousarray(tcoh[c * spc:(c + 1) * spc, None]),
            "rcnt_in": np.ascontiguousarray(
                (1.0 / np.maximum(d['counts'], 1.0))[:, None]),
        })
    return nc, in_maps, B


def _finish(results, B):
    total = 0.0
    for r in results:
        total += float(r["loss_out"].astype(np.float64).sum())
    return np.array(total / B, dtype=np.float32)


def kernel(embeddings, teacher_centroids, teacher_cohesion,
           member_indices, segment_ids, num_segments=None, **_ignored):
    nc, in_maps, B = _prepare(embeddings, teacher_centroids, teacher_cohesion,
                              member_indices, segment_ids)
    res = run_bass_kernel_spmd(nc, in_maps, core_ids=list(range(N_CORES)))
    return _finish(res.results, B)


def run_traced(embeddings, teacher_centroids, teacher_cohesion,
               member_indices, segment_ids, num_segments=None,
               tmpdir=None, **_ignored):
    """Like kernel() but with NTFF profiling; returns (loss, BassKernelResults)."""
    _install_ntff_hook()
    nc, in_maps, B = _prepare(embeddings, teacher_centroids, teacher_cohesion,
                              member_indices, segment_ids)
    res = run_bass_kernel_spmd(nc, in_maps, core_ids=list(range(N_CORES)),
                               trace=True, tmpdir=tmpdir)
    return _finish(res.results, B), res


def _install_ntff_hook():
    try:
        import antenv
        from trn_agent_boot.trn_boot import _ntff_profile_via_ctypes
    except ImportError:
        return
    if 'antenv.axon_hooks' in sys.modules:
        return
    hook = _ntff_profile_via_ctypes('/opt/axon/libaxon_pjrt.so')
    mod = types.ModuleType('antenv.axon_hooks')
    mod.get_axon_ntff_profile_hook = lambda: hook
    mod.set_axon_ntff_profile_hook = lambda h: None
    sys.modules['antenv.axon_hooks'] = mod
    antenv.axon_hooks = mod
